# revision 3
# baseline (speedup 1.0000x reference)
"""Causal self-attention Trainium2 Bass kernel.

Problem: B=4, T=2048, C=1024, H=16 heads, D=64. y = CausalSelfAttn(x) with fused
QKV projection, causal softmax, out projection (eval mode, dropout 0).

Sharding over 8 NeuronCores: core c -> batch b = c//2, head-group g = c%2
(8 heads per group). Each core computes its batch's QKV projection restricted to
its 8 heads (tensor-parallel column slice), full causal attention for those
heads, and a partial out-projection (row slice of w_out). Host sums the two
partials per batch and adds b_out.

Device layout highlights (per core):
- x arrives pre-transposed (xT [C, T]) so both the QK projection (transposed
  output qk^T [dims, T]) and the V projection (natural [T, dims]) read it as
  the contraction-major operand; no on-chip transposes anywhere.
- scores are computed transposed (s^T [tk, tq]) so softmax'd weights feed the
  AV matmul as the stationary operand directly.
- rowsums ride along the AV matmul via a ones column appended to v.
- causal masking is a post-exp multiply with 4 static 0/1 tiles (only the
  4 diagonal 128x512 blocks per (head, span) need it).
- projections run in float32r (full-rate fp32), attention in bf16.
"""

import os
import numpy as np
import ml_dtypes
from contextlib import ExitStack

import concourse.tile as tile
from concourse import bacc, mybir
from concourse import bass_utils
from concourse import library_config

B, T, C, H, D = 4, 2048, 1024, 16, 64
NCORES = 8
HL = 8            # heads per core
QKDIM = 2 * HL * D  # 1024 (q 512 | k 512)
VDIM = HL * D       # 512
NSPAN = T // 512    # 4 tq spans of 512
NTK = T // 128      # 16 tk tiles of 128

f32 = mybir.dt.float32
f32r = mybir.dt.float32r
bf16 = mybir.dt.bfloat16
EXPF = mybir.ActivationFunctionType.Exp

_cache = {}


def _emit(nc, tc, ctx, d):
    """Emit one full kernel body. d holds DRAM APs."""
    sb = ctx.enter_context(tc.tile_pool(name="sb", bufs=1))
    xpool = ctx.enter_context(tc.tile_pool(name="xspan", bufs=2))
    expool = ctx.enter_context(tc.tile_pool(name="expS", bufs=4))
    rrpool = ctx.enter_context(tc.tile_pool(name="rrow", bufs=2))
    rbpool = ctx.enter_context(tc.tile_pool(name="rbc", bufs=2))
    ypool = ctx.enter_context(tc.tile_pool(name="ysb", bufs=2))
    proj_ps = ctx.enter_context(tc.tile_pool(name="proj_ps", bufs=2, space="PSUM"))
    sc_ps = ctx.enter_context(tc.tile_pool(name="sc_ps", bufs=2, space="PSUM"))
    av_ps = ctx.enter_context(tc.tile_pool(name="av_ps", bufs=2, space="PSUM"))

    # ---- persistent tiles ----
    wqk_t = sb.tile([128, 8, QKDIM], f32r, tag="wqk")
    nc.sync.dma_start(wqk_t[:], d["wqkT"].rearrange("(c p) m -> p c m", p=128))
    wv_t = sb.tile([128, 8, VDIM], f32r, tag="wv")
    nc.sync.dma_start(wv_t[:], d["wvT"].rearrange("(c p) v -> p c v", p=128))
    wo_t = sb.tile([128, 4, C], bf16, tag="wo")
    nc.sync.dma_start(wo_t[:], d["woT"].rearrange("(k p) n -> p k n", p=128))
    masks_t = sb.tile([128, 4, 512], bf16, tag="masks")
    nc.sync.dma_start(masks_t[:], d["masks"].rearrange("i p f -> p i f"))
    bqk_t = sb.tile([128, 8], f32, tag="bqk")
    nc.sync.dma_start(bqk_t[:], d["bqk"].rearrange("(m p) -> p m", p=128))
    bv_row = sb.tile([1, VDIM], f32, tag="bv_row")
    nc.sync.dma_start(bv_row[:], d["bv"].rearrange("(o v) -> o v", o=1))
    bv_bc = sb.tile([128, VDIM], f32, tag="bv_bc")
    nc.gpsimd.partition_broadcast(bv_bc[:], bv_row[:])

    qk_t = sb.tile([128, 8, T], bf16, tag="qkT")        # qk^T: m-tiles 0-3 q, 4-7 k
    vp_t = sb.tile([128, NTK, HL, D + 1], bf16, tag="vp")  # v' with ones col
    nc.gpsimd.memset(vp_t[:, :, :, D : D + 1], 1.0)
    att_t = sb.tile([128, 4, T], bf16, tag="attout")    # attn out^T, pair-tiles

    for s in range(NSPAN):
        tq0 = 512 * s
        # ---- x span load (8 c-tiles [128, 512]) ----
        xs = xpool.tile([128, 8, 512], f32r, tag="xs")
        for c in range(8):
            nc.sync.dma_start(
                xs[:, c, :], d["xT"][128 * c : 128 * (c + 1), tq0 : tq0 + 512]
            )
        # ---- QK projection (pairs-first order) ----
        for m in (0, 4, 1, 5, 2, 6, 3, 7):
            pp = proj_ps.tile([128, 512], f32, tag="pp")
            for c in range(8):
                nc.tensor.matmul(
                    pp[:], wqk_t[:, c, 128 * m : 128 * (m + 1)], xs[:, c, :],
                    start=(c == 0), stop=(c == 7),
                )
            nc.vector.tensor_scalar_add(
                qk_t[:, m, tq0 : tq0 + 512], pp[:], bqk_t[:, m : m + 1]
            )
        # ---- V projection ----
        for tt in range(4):
            ti = 4 * s + tt
            pp = proj_ps.tile([128, 512], f32, tag="pp")
            for c in range(8):
                nc.tensor.matmul(
                    pp[:], xs[:, c, 128 * tt : 128 * (tt + 1)], wv_t[:, c, :],
                    start=(c == 0), stop=(c == 7),
                )
            nc.vector.tensor_add(
                vp_t[:, ti, :, 0:D],
                pp[:].rearrange("p (h v) -> p h v", h=HL),
                bv_bc[:].rearrange("p (h v) -> p h v", h=HL),
            )
        # ---- attention, head pairs interleaved ----
        ntk = 4 * s + 4          # valid tk tiles this span
        for hp in range(4):
            avs = {}
            for h in (2 * hp, 2 * hp + 1):
                avs[h] = av_ps.tile([128, 512], f32, tag="av", name=f"av{h}")
            for g2 in range(ntk // 2):
                exps = {}
                for h in (2 * hp, 2 * hp + 1):
                    po = 64 * (h % 2)
                    sc = sc_ps.tile([128, 2, 512], f32, tag="sc")
                    for i2 in range(2):
                        tk = 2 * g2 + i2
                        nc.tensor.matmul(
                            sc[:, i2, :],
                            qk_t[po : po + 64, 4 + hp, 128 * tk : 128 * (tk + 1)],
                            qk_t[po : po + 64, hp, tq0 : tq0 + 512],
                            start=True, stop=True,
                        )
                    ex = expool.tile([128, 2, 512], bf16, tag="ex")
                    nc.scalar.activation(ex[:], sc[:], EXPF, scale=0.125)
                    exps[h] = ex
                for h in (2 * hp, 2 * hp + 1):
                    ex = exps[h]
                    for i2 in range(2):
                        tk = 2 * g2 + i2
                        di = tk - 4 * s
                        if di >= 0:
                            nc.vector.tensor_mul(
                                ex[:, i2, :], ex[:, i2, :], masks_t[:, di, :]
                            )
                        nc.tensor.matmul(
                            avs[h][0:65, :], vp_t[:, tk, h, :], ex[:, i2, :],
                            start=(tk == 0), stop=(tk == ntk - 1),
                        )
            for h in (2 * hp, 2 * hp + 1):
                po = 64 * (h % 2)
                rr = rrpool.tile([1, 512], f32, tag="rr")
                nc.vector.reciprocal(rr[:], avs[h][64:65, :])
                rb = rbpool.tile([64, 512], f32, tag="rb")
                nc.gpsimd.partition_broadcast(rb[:], rr[:])
                nc.vector.tensor_mul(
                    att_t[po : po + 64, hp, tq0 : tq0 + 512], avs[h][0:64, :], rb[:]
                )
        # ---- out projection for this span ----
        for tt in range(4):
            ti = 4 * s + tt
            for ch in range(2):
                pp = proj_ps.tile([128, 512], f32, tag="pp")
                for kp in range(4):
                    nc.tensor.matmul(
                        pp[:],
                        att_t[:, kp, 128 * ti : 128 * (ti + 1)],
                        wo_t[:, kp, 512 * ch : 512 * (ch + 1)],
                        start=(kp == 0), stop=(kp == 3),
                    )
                ysb = ypool.tile([128, 512], f32, tag="ysb")
                nc.vector.tensor_copy(ysb[:], pp[:])
                nc.sync.dma_start(
                    d["y"][128 * ti : 128 * (ti + 1), 512 * ch : 512 * (ch + 1)],
                    ysb[:],
                )


def build(repeat=1):
    key = ("nc", repeat)
    if key in _cache:
        return _cache[key]
    nc = bacc.Bacc(
        "TRN2", target_bir_lowering=False, debug=False,
        enable_asserts=False, num_devices=NCORES,
    )
    d = {
        "xT": nc.dram_tensor("xT", [C, T], f32r, kind="ExternalInput").ap(),
        "wqkT": nc.dram_tensor("wqkT", [C, QKDIM], f32r, kind="ExternalInput").ap(),
        "wvT": nc.dram_tensor("wvT", [C, VDIM], f32r, kind="ExternalInput").ap(),
        "woT": nc.dram_tensor("woT", [VDIM, C], bf16, kind="ExternalInput").ap(),
        "bqk": nc.dram_tensor("bqk", [QKDIM], f32, kind="ExternalInput").ap(),
        "bv": nc.dram_tensor("bv", [VDIM], f32, kind="ExternalInput").ap(),
        "masks": nc.dram_tensor("masks", [4, 128, 512], bf16, kind="ExternalInput").ap(),
        "y": nc.dram_tensor("y", [T, C], f32, kind="ExternalOutput").ap(),
    }
    with tile.TileContext(nc) as tc:
        with ExitStack() as ctx:
            nc.gpsimd.load_library(library_config.attn)
            if repeat > 1:
                with tc.For_i(0, repeat, 1):
                    _emit(nc, tc, ctx, d)
            else:
                _emit(nc, tc, ctx, d)
    nc.compile()
    _cache[key] = nc
    return nc


def make_masks():
    m = np.zeros((4, 128, 512), np.float32)
    f = np.arange(512)[None, :]
    p = np.arange(128)[:, None]
    for i in range(4):
        m[i] = (f >= 128 * i + p).astype(np.float32)
    return m.astype(ml_dtypes.bfloat16)


def make_in_maps(x, w_qkv, b_qkv, w_out):
    masks = make_masks()
    in_maps = []
    for c in range(NCORES):
        b, g = c // 2, c % 2
        wq = w_qkv[512 * g : 512 * (g + 1)]
        wk = w_qkv[C + 512 * g : C + 512 * (g + 1)]
        wv = w_qkv[2 * C + 512 * g : 2 * C + 512 * (g + 1)]
        in_maps.append({
            "xT": np.ascontiguousarray(x[b].T),
            "wqkT": np.ascontiguousarray(np.concatenate([wq, wk], 0).T),
            "wvT": np.ascontiguousarray(wv.T),
            "woT": np.ascontiguousarray(w_out[:, 512 * g : 512 * (g + 1)].T).astype(
                ml_dtypes.bfloat16
            ),
            "bqk": np.ascontiguousarray(
                np.concatenate([b_qkv[512 * g : 512 * (g + 1)],
                                b_qkv[C + 512 * g : C + 512 * (g + 1)]])
            ),
            "bv": np.ascontiguousarray(b_qkv[2 * C + 512 * g : 2 * C + 512 * (g + 1)]),
            "masks": masks,
        })
    return in_maps


def kernel(x, w_qkv, b_qkv, w_out, b_out):
    x = np.asarray(x, np.float32)
    w_qkv = np.asarray(w_qkv, np.float32)
    b_qkv = np.asarray(b_qkv, np.float32)
    w_out = np.asarray(w_out, np.float32)
    b_out = np.asarray(b_out, np.float32)

    nc = build()
    in_maps = make_in_maps(x, w_qkv, b_qkv, w_out)
    for attempt in range(3):
        res = bass_utils.run_bass_kernel_spmd(
            nc, in_maps, core_ids=list(range(NCORES))
        )
        parts = [res.results[c]["y"] for c in range(NCORES)]
        y = np.empty((B, T, C), np.float32)
        for b in range(B):
            y[b] = parts[2 * b] + parts[2 * b + 1] + b_out
        # guard against a rare transport glitch returning all-zero buffers
        if all(np.abs(p).max() > 0 for p in parts):
            break
    return y


# revision 7
# speedup vs baseline: 1.0039x; 1.0039x over previous
"""Causal self-attention Trainium2 Bass kernel.

Problem: B=4, T=2048, C=1024, H=16 heads, D=64. y = CausalSelfAttn(x) with fused
QKV projection, causal softmax, out projection (eval mode, dropout 0).

Sharding over 8 NeuronCores: core c -> batch b = c//2, head-group g = c%2
(8 heads per group). Each core computes its batch's QKV projection restricted to
its 8 heads (tensor-parallel column slice), full causal attention for those
heads, and a partial out-projection (row slice of w_out). Host sums the two
partials per batch and adds b_out.

Device layout highlights (per core):
- x arrives pre-transposed (xT [C, T]) so both the QK projection (transposed
  output qk^T [dims, T]) and the V projection (natural [T, dims]) read it as
  the contraction-major operand; no on-chip transposes anywhere.
- scores are computed transposed (s^T [tk, tq]) so softmax'd weights feed the
  AV matmul as the stationary operand directly.
- rowsums ride along the AV matmul via a ones column appended to v.
- causal masking is a post-exp multiply with 4 static 0/1 tiles (only the
  4 diagonal 128x512 blocks per (head, span) need it).
- projections run in float32r (full-rate fp32), attention in bf16.
"""

import os
import numpy as np
import ml_dtypes
from contextlib import ExitStack

import concourse.tile as tile
from concourse import bacc, mybir
from concourse import bass_utils
from concourse import library_config

B, T, C, H, D = 4, 2048, 1024, 16, 64
NCORES = 8
HL = 8            # heads per core
QKDIM = 2 * HL * D  # 1024 (q 512 | k 512)
VDIM = HL * D       # 512
NSPAN = T // 512    # 4 tq spans of 512
NTK = T // 128      # 16 tk tiles of 128

f32 = mybir.dt.float32
f32r = mybir.dt.float32r
bf16 = mybir.dt.bfloat16
EXPF = mybir.ActivationFunctionType.Exp

_cache = {}
LABELS = {}


def _lab(mi, label):
    try:
        LABELS[mi.ins.name] = label
    except Exception:
        pass
    return mi


def _emit(nc, tc, ctx, d):
    """Emit one full kernel body. d holds DRAM APs."""
    sb = ctx.enter_context(tc.tile_pool(name="sb", bufs=1))
    xpool = ctx.enter_context(tc.tile_pool(name="xspan", bufs=2))
    expool = ctx.enter_context(tc.tile_pool(name="expS", bufs=4))
    rrpool = ctx.enter_context(tc.tile_pool(name="rrow", bufs=2))
    rbpool = ctx.enter_context(tc.tile_pool(name="rbc", bufs=2))
    ypool = ctx.enter_context(tc.tile_pool(name="ysb", bufs=2))
    proj_ps = ctx.enter_context(tc.tile_pool(name="proj_ps", bufs=2, space="PSUM"))
    sc_ps = ctx.enter_context(tc.tile_pool(name="sc_ps", bufs=2, space="PSUM"))
    av_ps = ctx.enter_context(tc.tile_pool(name="av_ps", bufs=2, space="PSUM"))

    # ---- persistent tiles (DMA split per c-tile for fine-grained deps) ----
    bqk_t = sb.tile([128, 8], f32, tag="bqk")
    nc.sync.dma_start(bqk_t[:], d["bqk"].rearrange("(m p) -> p m", p=128))
    bv_row = sb.tile([1, VDIM], f32, tag="bv_row")
    nc.sync.dma_start(bv_row[:], d["bv"].rearrange("(o v) -> o v", o=1))
    bv_bc = sb.tile([128, VDIM], f32, tag="bv_bc")
    nc.gpsimd.partition_broadcast(bv_bc[:], bv_row[:])
    wqk_t = sb.tile([128, 8, QKDIM], f32r, tag="wqk")
    wqk_r = d["wqkT"].rearrange("(c p) m -> p c m", p=128)
    wv_t = sb.tile([128, 8, VDIM], f32r, tag="wv")
    wv_r = d["wvT"].rearrange("(c p) v -> p c v", p=128)
    for c in range(8):
        nc.sync.dma_start(wqk_t[:, c, :], wqk_r[:, c, :])
    for c in range(8):
        nc.sync.dma_start(wv_t[:, c, :], wv_r[:, c, :])
    wo_t = sb.tile([128, 4, C], bf16, tag="wo")
    masks_t = sb.tile([128, 4, 512], bf16, tag="masks")

    qk_t = sb.tile([128, 8, T], bf16, tag="qkT")        # qk^T: m-tiles 0-3 q, 4-7 k
    vp_t = sb.tile([128, NTK, HL, D + 1], bf16, tag="vp")  # v' with ones col
    nc.gpsimd.memset(vp_t[:, :, :, D : D + 1], 1.0)
    att_t = sb.tile([128, 4, T], bf16, tag="attout")    # attn out^T, pair-tiles

    def proj_span(s):
        tq0 = 512 * s
        # x span load (8 c-tiles [128, 512])
        xs = xpool.tile([128, 8, 512], f32r, tag="xs", name=f"xs{s}")
        for c in range(8):
            nc.sync.dma_start(
                xs[:, c, :], d["xT"][128 * c : 128 * (c + 1), tq0 : tq0 + 512]
            )
        if s == 0:
            nc.sync.dma_start(
                masks_t[:], d["masks"].rearrange("i p f -> p i f"))
            nc.sync.dma_start(
                wo_t[:], d["woT"].rearrange("(k p) n -> p k n", p=128))
        # QK projection (pairs-first order)
        for m in (0, 4, 1, 5, 2, 6, 3, 7):
            pp = proj_ps.tile([128, 512], f32, tag="pp", name=f"pq{s}{m}")
            for c in range(8):
                _lab(nc.tensor.matmul(
                    pp[:], wqk_t[:, c, 128 * m : 128 * (m + 1)], xs[:, c, :],
                    start=(c == 0), stop=(c == 7),
                ), f"qkproj_s{s}_m{m}")
            nc.vector.tensor_scalar_add(
                qk_t[:, m, tq0 : tq0 + 512], pp[:], bqk_t[:, m : m + 1]
            )
        # V projection
        for tt in range(4):
            ti = 4 * s + tt
            pp = proj_ps.tile([128, 512], f32, tag="pp", name=f"pv{s}{tt}")
            for c in range(8):
                _lab(nc.tensor.matmul(
                    pp[:], xs[:, c, 128 * tt : 128 * (tt + 1)], wv_t[:, c, :],
                    start=(c == 0), stop=(c == 7),
                ), f"vproj_s{s}_t{tt}")
            nc.vector.tensor_add(
                vp_t[:, ti, :, 0:D],
                pp[:].rearrange("p (h v) -> p h v", h=HL),
                bv_bc[:].rearrange("p (h v) -> p h v", h=HL),
            )

    def attn_span(s):
        tq0 = 512 * s
        ntk = 4 * s + 4          # valid tk tiles this span
        for hp in range(4):
            avs = {}
            for h in (2 * hp, 2 * hp + 1):
                avs[h] = av_ps.tile([128, 512], f32, tag="av", name=f"av{h}")
            for g2 in range(ntk // 2):
                exps = {}
                for h in (2 * hp, 2 * hp + 1):
                    po = 64 * (h % 2)
                    sc = sc_ps.tile([128, 2, 512], f32, tag="sc")
                    for i2 in range(2):
                        tk = 2 * g2 + i2
                        _lab(nc.tensor.matmul(
                            sc[:, i2, :],
                            qk_t[po : po + 64, 4 + hp, 128 * tk : 128 * (tk + 1)],
                            qk_t[po : po + 64, hp, tq0 : tq0 + 512],
                            start=True, stop=True,
                        ), f"score_s{s}_h{h}_tk{2*g2+i2}")
                    ex = expool.tile([128, 2, 512], bf16, tag="ex")
                    nc.scalar.activation(ex[:], sc[:], EXPF, scale=0.125)
                    exps[h] = ex
                for h in (2 * hp, 2 * hp + 1):
                    ex = exps[h]
                    for i2 in range(2):
                        tk = 2 * g2 + i2
                        di = tk - 4 * s
                        if di >= 0:
                            nc.vector.tensor_mul(
                                ex[:, i2, :], ex[:, i2, :], masks_t[:, di, :]
                            )
                        _lab(nc.tensor.matmul(
                            avs[h][0:65, :], vp_t[:, tk, h, :], ex[:, i2, :],
                            start=(tk == 0), stop=(tk == ntk - 1),
                        ), f"av_s{s}_h{h}_tk{tk}")
            for h in (2 * hp, 2 * hp + 1):
                po = 64 * (h % 2)
                rr = rrpool.tile([1, 512], f32, tag="rr")
                nc.vector.reciprocal(rr[:], avs[h][64:65, :])
                rb = rbpool.tile([64, 512], f32, tag="rb")
                nc.gpsimd.partition_broadcast(rb[:], rr[:])
                nc.vector.tensor_mul(
                    att_t[po : po + 64, hp, tq0 : tq0 + 512], avs[h][0:64, :], rb[:]
                )

    def oproj_span(s):
        for tt in range(4):
            ti = 4 * s + tt
            for ch in range(2):
                pp = proj_ps.tile([128, 512], f32, tag="pp", name=f"po{s}{tt}{ch}")
                for kp in range(4):
                    _lab(nc.tensor.matmul(
                        pp[:],
                        att_t[:, kp, 128 * ti : 128 * (ti + 1)],
                        wo_t[:, kp, 512 * ch : 512 * (ch + 1)],
                        start=(kp == 0), stop=(kp == 3),
                    ), f"oproj_s{s}_t{tt}_ch{ch}_k{kp}")
                ysb = ypool.tile([128, 512], f32, tag="ysb", name=f"y{s}{tt}{ch}")
                nc.vector.tensor_copy(ysb[:], pp[:])
                nc.sync.dma_start(
                    d["y"][128 * ti : 128 * (ti + 1), 512 * ch : 512 * (ch + 1)],
                    ysb[:],
                )

    # schedule: attention(s) overlaps projection(s+1); out-proj(s) fills the
    # tail while the next span's attention waits on its exps.
    proj_span(0)
    for s in range(NSPAN):
        attn_span(s)
        if s + 1 < NSPAN:
            proj_span(s + 1)
        oproj_span(s)


def build(repeat=1):
    key = ("nc", repeat)
    if key in _cache:
        return _cache[key]
    nc = bacc.Bacc(
        "TRN2", target_bir_lowering=False, debug=False,
        enable_asserts=False, num_devices=NCORES,
    )
    d = {
        "xT": nc.dram_tensor("xT", [C, T], f32r, kind="ExternalInput").ap(),
        "wqkT": nc.dram_tensor("wqkT", [C, QKDIM], f32r, kind="ExternalInput").ap(),
        "wvT": nc.dram_tensor("wvT", [C, VDIM], f32r, kind="ExternalInput").ap(),
        "woT": nc.dram_tensor("woT", [VDIM, C], bf16, kind="ExternalInput").ap(),
        "bqk": nc.dram_tensor("bqk", [QKDIM], f32, kind="ExternalInput").ap(),
        "bv": nc.dram_tensor("bv", [VDIM], f32, kind="ExternalInput").ap(),
        "masks": nc.dram_tensor("masks", [4, 128, 512], bf16, kind="ExternalInput").ap(),
        "y": nc.dram_tensor("y", [T, C], f32, kind="ExternalOutput").ap(),
    }
    with tile.TileContext(nc) as tc:
        with ExitStack() as ctx:
            nc.gpsimd.load_library(library_config.attn)
            if repeat > 1:
                with tc.For_i(0, repeat, 1):
                    _emit(nc, tc, ctx, d)
            else:
                _emit(nc, tc, ctx, d)
    nc.compile()
    _cache[key] = nc
    return nc


def make_masks():
    m = np.zeros((4, 128, 512), np.float32)
    f = np.arange(512)[None, :]
    p = np.arange(128)[:, None]
    for i in range(4):
        m[i] = (f >= 128 * i + p).astype(np.float32)
    return m.astype(ml_dtypes.bfloat16)


def make_in_maps(x, w_qkv, b_qkv, w_out):
    masks = make_masks()
    in_maps = []
    for c in range(NCORES):
        b, g = c // 2, c % 2
        wq = w_qkv[512 * g : 512 * (g + 1)]
        wk = w_qkv[C + 512 * g : C + 512 * (g + 1)]
        wv = w_qkv[2 * C + 512 * g : 2 * C + 512 * (g + 1)]
        in_maps.append({
            "xT": np.ascontiguousarray(x[b].T),
            "wqkT": np.ascontiguousarray(np.concatenate([wq, wk], 0).T),
            "wvT": np.ascontiguousarray(wv.T),
            "woT": np.ascontiguousarray(w_out[:, 512 * g : 512 * (g + 1)].T).astype(
                ml_dtypes.bfloat16
            ),
            "bqk": np.ascontiguousarray(
                np.concatenate([b_qkv[512 * g : 512 * (g + 1)],
                                b_qkv[C + 512 * g : C + 512 * (g + 1)]])
            ),
            "bv": np.ascontiguousarray(b_qkv[2 * C + 512 * g : 2 * C + 512 * (g + 1)]),
            "masks": masks,
        })
    return in_maps


def kernel(x, w_qkv, b_qkv, w_out, b_out):
    x = np.asarray(x, np.float32)
    w_qkv = np.asarray(w_qkv, np.float32)
    b_qkv = np.asarray(b_qkv, np.float32)
    w_out = np.asarray(w_out, np.float32)
    b_out = np.asarray(b_out, np.float32)

    nc = build()
    in_maps = make_in_maps(x, w_qkv, b_qkv, w_out)
    for attempt in range(3):
        res = bass_utils.run_bass_kernel_spmd(
            nc, in_maps, core_ids=list(range(NCORES))
        )
        parts = [res.results[c]["y"] for c in range(NCORES)]
        y = np.empty((B, T, C), np.float32)
        for b in range(B):
            y[b] = parts[2 * b] + parts[2 * b + 1] + b_out
        # guard against a rare transport glitch returning all-zero buffers
        if all(np.abs(p).max() > 0 for p in parts):
            break
    return y
